# revision 15
# baseline (speedup 1.0000x reference)
"""Trainium2 Bass kernel for nn_AlgelogicNetwork (fuzzy rule matching -> softmax).

kernel(**inputs) takes the FULL unsharded inputs of reference.setup_inputs()
and returns the FULL output (softmax over M=16 rule strengths, (16,) float32).

The problem is tiny (<<1MB), so the whole computation is replicated on each of
the 8 NeuronCores (SPMD with identical inputs); core 0's output is returned.

v2 design (m = 16 partitions everywhere, minimal serial op count):
  - Host packs all inputs into ONE [16, 120] f32 array (layout ops only:
    reshape/transpose/tile). c and wm are pre-tiled to the flat 36-slot
    (w, j, l) layout so every elementwise op uses <= 2 free dims (HW limit
    for TensorTensor/ScalarTensorTensor); head_w is packed (i,j,l)-ordered so
    the (g>.5) mask broadcast merges into a contiguous jl dim.
  - match~[m,j,w] = sum_l sig[m,j,l]*(c-wm)^2 on Pool: d = crep - wmrep
    (flat), d2 = d*d, ws = d2 * sig (bcast w), match = ws[l=0]+ws[l=1].
  - hww[m,i,j,w] = sum_l (g>.5)*head*wm on DVE; (g>.5)*head is ONE fused
    scalar_tensor_tensor (is_gt then mult).
  - argmin one-hot + gather fused per premise j: scalar_tensor_tensor with the
    per-partition min_j as the scalar operand; j=0 on DVE, j=1 on Pool run in
    parallel; cap[m,i] then falls out of ONE reduce over (j,w) (AxisList XY).
  - tail Linear with bias via [tail|bias] x [cap|1] and a stt+accum_out for
    P2 = |concl|^2; softmax stays in per-partition column form: ACT Sqrt,
    ACT Exp, then S is broadcast to all 16 partitions by a single PE matmul
    whose stationary is the e-column with free-stride 0 (out[a,b] =
    sum_p e[p]*ones[p] = S for every a), recip + mul on DVE, and the output
    DMA gathers the [16,1] column into y[1,16].
  - Bass-init const-AP memsets and the entry all-engine barrier are patched
    out (see _make_bass) so the input DMA issues at ~300ns instead of ~1030ns.
  - Every op keeps the intra-engine own-semaphore discipline for dependent
    same-engine pairs (HW-validated requirement).
"""
import numpy as np
import concourse.bass as bass
from concourse import mybir

F32 = mybir.dt.float32
M, J, I, L, W = 16, 2, 3, 2, 9
FREE = 640
NPACK = 120

# packed columns
C_G, C_HEADI, C_WREP, C_TAILX = 0, 4, 16, 34
C_CAPX, C_ONE, C_BM5, C_ZERO = 42, 45, 46, 47
C_CREP, C_WMR = 48, 84
# computed columns
C_SIG = 120
C_HM = 124
C_T1, C_T2, C_HWW = 136, 190, 244
C_D, C_D2, C_WS = 300, 336, 372
C_MATCH, C_MIN = 408, 426
C_PSEL = 428
C_QA0, C_QA1, C_Q, C_Z, C_ZJ = 482, 498, 514, 530, 546
C_P2, C_P, C_E, C_SINV, C_OUT = 562, 563, 564, 565, 566


def pack_inputs(state, constants, gammas, head_w, tail_w, tail_b):
    p = np.zeros((M, NPACK), np.float32)
    wm = np.asarray(state, np.float32).reshape(W, L)
    p[:, C_G:C_G + 4] = gammas[:, 1:1 + J, :].reshape(M, J * L)
    # head reordered (i, j, l)
    p[:, C_HEADI:C_HEADI + 12] = head_w.transpose(0, 2, 1, 3).reshape(M, I * J * L)
    te = np.concatenate([tail_w, tail_b[:, :, None]], axis=2)        # [16,2,4]
    p[:, C_TAILX:C_TAILX + 8] = te.reshape(M, L * 4)
    p[:, C_ONE] = 1.0
    p[:, C_BM5] = -5.0
    p[:, C_ZERO] = 0.0
    # crep[m, w*4 + j*2 + l] = constants[m, j, l]  (tiled over w)
    c4 = constants[:, :J, :].reshape(M, J * L)
    p[:, C_CREP:C_CREP + 36] = np.tile(c4, (1, W))
    # wmrep36[m, w*4 + j*2 + l] = wm[w, l]  (tiled over j)
    wmr = np.repeat(wm, J, axis=0).reshape(1, W * J * L)             # w,j,l
    p[:, C_WMR:C_WMR + 36] = np.tile(wmr, (M, 1))
    return p


def _make_bass():
    """Bass() with the init-time const-AP memsets and entry all-engine barrier
    suppressed. The four const-AP memsets ([128,1] gpsimd memsets of 0.0/1.0/
    bf16-1.0/u8-127) are only consulted when an activation bias/scale is passed
    as a float immediate - this kernel always passes AP biases, so the const
    APs are never read. The entry barrier only orders engine preambles against
    user code; every op here gates on the input-DMA semaphore (incremented by
    SP after its own preamble), so the barrier is redundant. The Block-exit
    barrier is unaffected (patches are restored before Block exit)."""
    orig_memset = bass.BassSharedVectorInterface.memset
    orig_barrier = bass.Bass.all_engine_barrier
    bass.BassSharedVectorInterface.memset = lambda self, ap, c: None
    bass.Bass.all_engine_barrier = lambda self, sem_only=False: None
    try:
        nc = bass.Bass("TRN2", target_bir_lowering=False, debug=False)
    finally:
        bass.BassSharedVectorInterface.memset = orig_memset
        bass.Bass.all_engine_barrier = orig_barrier
    return nc


def build():
    nc = _make_bass()
    packed = nc.dram_tensor("packed", [M, NPACK], F32, kind="ExternalInput")
    y = nc.dram_tensor("y", [1, 16], F32, kind="ExternalOutput")

    al = mybir.AluOpType
    af = mybir.ActivationFunctionType

    with (
        nc.sbuf_tensor("sb", [128, FREE], F32) as sb,
        nc.psum_tensor("S", [16, 1], F32) as ps,
        nc.semaphore("s_dma") as s_dma,
        nc.semaphore("s_act") as s_act,
        nc.semaphore("s_dve") as s_dve,
        nc.semaphore("s_pe") as s_pe,
        nc.semaphore("s_out") as s_out,
        nc.semaphore("s_pool") as s_pool,
    ):
        def A(r0, nr, c0, dims):
            return bass.AP(sb, r0 * FREE + c0, [[FREE, nr]] + [list(d) for d in dims])

        SPS = lambda: bass.AP(ps, 0, [[1, 16], [1, 1]])

        sems = {"ACT": s_act, "DVE": s_dve, "PE": s_pe, "DMA": s_dma,
                "OUT": s_out, "POOL": s_pool}
        counts = {"ACT": 0, "DVE": 0, "PE": 0, "POOL": 0}
        waited = {k: {} for k in ("ACT", "DVE", "PE", "SP", "POOL")}

        def emit(ekey, engine, build_fn, deps=(), inc=True, own=True):
            # Intra-engine semaphore waits are REQUIRED on this hardware for
            # every DEPENDENT same-engine pair (HW-tested: dropping them
            # corrupts outputs). own=False is legal only when the previous
            # same-engine op is data-independent (disjoint regions; in-order
            # execution suffices) or its completion is transitively implied
            # by one of this op's cross-engine waits (vector-clock join).
            need = {}
            if own and ekey in counts and counts[ekey] > 0:
                need[ekey] = counts[ekey]
            for sk, v in deps:
                if sk == ekey:
                    continue
                need[sk] = max(need.get(sk, 0), v)
            fresh = [(sk, v) for sk, v in need.items() if waited[ekey].get(sk, 0) < v]
            for sk, v in fresh[1:]:
                engine.wait_ge(sems[sk], v)
            inst = build_fn()
            for sk, v in fresh[:1]:
                inst._wait_ge(sems[sk], v)
            for sk, v in fresh:
                waited[ekey][sk] = v
            if inc and ekey in counts:
                counts[ekey] += 1
                inst.then_inc(sems[ekey], 1)
            return inst

        with nc.Block() as block:

            @block.sync
            def _(sync):
                sync.dma_start(
                    out=A(0, M, 0, [(1, NPACK)]),
                    in_=bass.AP(packed, 0, [[NPACK, M], [1, NPACK]]),
                ).then_inc(s_dma, 16)

            @block.scalar
            def _(scalar):
                # a1: sig = sigmoid(10*g - 5)   [16,(j,l)]
                emit("ACT", scalar, lambda: scalar.activation(
                    A(0, M, C_SIG, [(1, 4)]), A(0, M, C_G, [(1, 4)]),
                    af.Sigmoid, bias=A(0, M, C_BM5, [(1, 1)]), scale=10.0,
                ), deps=[("DMA", 16)])

            @block.gpsimd
            def _(g):
                # q0: qa0 = tailx (x) tailx  for l=0   [16,(a,b)]
                emit("POOL", g, lambda: g.tensor_mul(
                    A(0, M, C_QA0, [(4, 4), (1, 4)]),
                    A(0, M, C_TAILX, [(1, 4), (0, 4)]),
                    A(0, M, C_TAILX, [(0, 4), (1, 4)]),
                ), deps=[("DMA", 16)])
                # t1 = hm[l=0] * wm[l=0]   [16,(ij,w)]
                emit("POOL", g, lambda: g.tensor_mul(
                    A(0, M, C_T1, [(9, 6), (1, 9)]),
                    A(0, M, C_HM, [(2, 6), (0, 9)]),
                    A(0, M, C_WMR, [(0, 6), (4, 9)]),
                ), deps=[("DVE", 2)], own=False)
                # t2 = hm[l=1] * wm[l=1]  (independent of t1; in-order)
                emit("POOL", g, lambda: g.tensor_mul(
                    A(0, M, C_T2, [(9, 6), (1, 9)]),
                    A(0, M, C_HM + 1, [(2, 6), (0, 9)]),
                    A(0, M, C_WMR + 1, [(0, 6), (4, 9)]),
                ), own=False)
                # hww = t1 + t2   [16,(i,j,w)]
                emit("POOL", g, lambda: g.tensor_add(
                    A(0, M, C_HWW, [(1, 54)]), A(0, M, C_T1, [(1, 54)]),
                    A(0, M, C_T2, [(1, 54)]),
                ))
                # qa1 (independent of hww; in-order after it)
                emit("POOL", g, lambda: g.tensor_mul(
                    A(0, M, C_QA1, [(4, 4), (1, 4)]),
                    A(0, M, C_TAILX + 4, [(1, 4), (0, 4)]),
                    A(0, M, C_TAILX + 4, [(0, 4), (1, 4)]),
                ), own=False)
                # Q = qa0 + qa1 = sum_l tailx_l (x) tailx_l   [16,(a,b)]
                emit("POOL", g, lambda: g.tensor_add(
                    A(0, M, C_Q, [(1, 16)]), A(0, M, C_QA0, [(1, 16)]),
                    A(0, M, C_QA1, [(1, 16)]),
                ))

            @block.vector
            def _(vector):
                # v1: d = crep - wmrep   [16, 36]
                emit("DVE", vector, lambda: vector.tensor_tensor(
                    A(0, M, C_D, [(1, 36)]), A(0, M, C_CREP, [(1, 36)]),
                    A(0, M, C_WMR, [(1, 36)]), al.subtract,
                ), deps=[("DMA", 16)])
                # v2: hm = (g > 0.5) * head   [16,(i,jl)]  (indep of d)
                emit("DVE", vector, lambda: vector.scalar_tensor_tensor(
                    A(0, M, C_HM, [(4, 3), (1, 4)]),
                    A(0, M, C_G, [(0, 3), (1, 4)]),
                    0.5,
                    A(0, M, C_HEADI, [(4, 3), (1, 4)]),
                    op0=al.is_gt, op1=al.mult,
                ), own=False)
                # v3: d2 = d*d  (dep d = inc 1 only, not hm)
                emit("DVE", vector, lambda: vector.tensor_mul(
                    A(0, M, C_D2, [(1, 36)]), A(0, M, C_D, [(1, 36)]),
                    A(0, M, C_D, [(1, 36)]),
                ), deps=[("DVE", 1)], own=False)
                # v4: ws = d2 * sig (bcast w; jl contiguous)
                emit("DVE", vector, lambda: vector.tensor_mul(
                    A(0, M, C_WS, [(4, 9), (1, 4)]),
                    A(0, M, C_D2, [(4, 9), (1, 4)]),
                    A(0, M, C_SIG, [(0, 9), (1, 4)]),
                ), deps=[("ACT", 1)])
                # v5: match = ws[l=0] + ws[l=1]   [16,(j,w)]
                emit("DVE", vector, lambda: vector.tensor_add(
                    A(0, M, C_MATCH, [(9, 2), (1, 9)]),
                    A(0, M, C_WS, [(2, 2), (4, 9)]),
                    A(0, M, C_WS + 1, [(2, 2), (4, 9)]),
                ))
                # v6: min over w per j    [16,(j)]
                emit("DVE", vector, lambda: vector.tensor_reduce(
                    A(0, M, C_MIN, [(1, 2)]),
                    A(0, M, C_MATCH, [(9, 2), (1, 9)]),
                    axis=mybir.AxisListType.X, op=al.min,
                ))
                # v7: psel_j0 = (match_0 == min_0) * hww_0   [16,(i,w)]
                emit("DVE", vector, lambda: vector.scalar_tensor_tensor(
                    A(0, M, C_PSEL, [(18, 3), (1, 9)]),
                    A(0, M, C_MATCH, [(0, 3), (1, 9)]),
                    A(0, M, C_MIN, [(1, 1)]),
                    A(0, M, C_HWW, [(18, 3), (1, 9)]),
                    op0=al.is_equal, op1=al.mult,
                ), deps=[("POOL", 4)])
                # v8: psel_j1 (deps covered by v7's waits; in-order)
                emit("DVE", vector, lambda: vector.scalar_tensor_tensor(
                    A(0, M, C_PSEL + 9, [(18, 3), (1, 9)]),
                    A(0, M, C_MATCH + 9, [(0, 3), (1, 9)]),
                    A(0, M, C_MIN + 1, [(1, 1)]),
                    A(0, M, C_HWW + 9, [(18, 3), (1, 9)]),
                    op0=al.is_equal, op1=al.mult,
                ), own=False)
                # v9: cap = sum_{j,w} psel   [16,(i)]
                emit("DVE", vector, lambda: vector.tensor_reduce(
                    A(0, M, C_CAPX, [(1, 3)]),
                    A(0, M, C_PSEL, [(18, 3), (9, 2), (1, 9)]),
                    axis=mybir.AxisListType.XY, op=al.add,
                ))
                # v10: z = capx (x) capx   [16,(a,b)]
                emit("DVE", vector, lambda: vector.tensor_mul(
                    A(0, M, C_Z, [(4, 4), (1, 4)]),
                    A(0, M, C_CAPX, [(1, 4), (0, 4)]),
                    A(0, M, C_CAPX, [(0, 4), (1, 4)]),
                ))
                # v11: P2 = sum_{a,b} z*Q  (= |T capx|^2, bias included)
                emit("DVE", vector, lambda: vector.scalar_tensor_tensor(
                    A(0, M, C_ZJ, [(1, 16)]), A(0, M, C_Z, [(1, 16)]), 1.0,
                    A(0, M, C_Q, [(1, 16)]), op0=al.mult, op1=al.mult,
                    accum_out=A(0, M, C_P2, [(1, 1)]),
                ), deps=[("POOL", 6)])

            @block.scalar
            def _(scalar):
                # a2: P = sqrt(P2)   [16,1]
                emit("ACT", scalar, lambda: scalar.activation(
                    A(0, M, C_P, [(1, 1)]), A(0, M, C_P2, [(1, 1)]),
                    af.Sqrt, bias=A(0, M, C_ZERO, [(1, 1)]), scale=1.0,
                ), deps=[("DVE", 11)], own=False)
                # a3: e = exp(P)   [16,1]
                emit("ACT", scalar, lambda: scalar.activation(
                    A(0, M, C_E, [(1, 1)]), A(0, M, C_P, [(1, 1)]),
                    af.Exp, bias=A(0, M, C_ZERO, [(1, 1)]), scale=1.0,
                ))

            @block.tensor
            def _(tensor):
                # m1: S = sum_p e[p], broadcast to 16 partitions via
                # free-stride-0 stationary: out[a,0] = sum_p e[p]*one[p]
                emit("PE", tensor, lambda: tensor.matmul(
                    SPS(),
                    bass.AP(sb, C_E, [[FREE, 16], [0, 16]]),
                    A(0, 16, C_ONE, [(1, 1)]),
                    start=True, stop=True,
                ), deps=[("ACT", 3)])

            @block.vector
            def _(vector):
                # v12: sinv = 1/S (PSUM -> SBUF)
                emit("DVE", vector, lambda: vector.reciprocal(
                    A(0, M, C_SINV, [(1, 1)]), SPS(),
                ), deps=[("PE", 1)], own=False)
                # v13: out = e * sinv
                emit("DVE", vector, lambda: vector.tensor_scalar(
                    A(0, M, C_OUT, [(1, 1)]), A(0, M, C_E, [(1, 1)]),
                    A(0, M, C_SINV, [(1, 1)]), None, al.mult,
                ))

            @block.sync
            def _(sync):
                emit("SP", sync, lambda: sync.dma_start(
                    out=bass.AP(y, 0, [[16, 1], [1, 16]]),
                    in_=A(0, M, C_OUT, [(1, 1)]),
                ), deps=[("DVE", 13)], inc=False).then_inc(s_out, 16)

    return nc


_NC = None


def _get_nc():
    global _NC
    if _NC is None:
        _NC = build()
    return _NC


def _default_inputs():
    """Regenerate setup_inputs()'s non-state parameters (jax key(0) recipe) in
    case the harness only supplies `state` (spec.json lists only state in
    input_specs)."""
    import jax
    import jax.numpy as jnp
    key = jax.random.key(0)
    ks = jax.random.split(key, 6)
    bL = 1.0 / np.sqrt(L)
    bI = 1.0 / np.sqrt(I)
    return dict(
        state=jax.random.normal(ks[0], (1, W * L), dtype=jnp.float32),
        constants=jax.random.uniform(ks[1], (M, J + 1, L), minval=-1.0, maxval=1.0, dtype=jnp.float32),
        gammas=jax.random.uniform(ks[2], (M, J + 1, L), minval=0.0, maxval=1.0, dtype=jnp.float32),
        head_w=jax.random.uniform(ks[3], (M, J, I, L), minval=-bL, maxval=bL, dtype=jnp.float32),
        tail_w=jax.random.uniform(ks[4], (M, L, I), minval=-bI, maxval=bI, dtype=jnp.float32),
        tail_b=jax.random.uniform(ks[5], (M, L), minval=-bI, maxval=bI, dtype=jnp.float32),
    )


def kernel(state=None, constants=None, gammas=None, head_w=None, tail_w=None,
           tail_b=None, **_unused):
    from concourse.bass_utils import run_bass_kernel_spmd

    if any(v is None for v in (state, constants, gammas, head_w, tail_w, tail_b)):
        d = _default_inputs()
        state = d["state"] if state is None else state
        constants = d["constants"] if constants is None else constants
        gammas = d["gammas"] if gammas is None else gammas
        head_w = d["head_w"] if head_w is None else head_w
        tail_w = d["tail_w"] if tail_w is None else tail_w
        tail_b = d["tail_b"] if tail_b is None else tail_b

    state = np.asarray(state, np.float32)
    constants = np.asarray(constants, np.float32)
    gammas = np.asarray(gammas, np.float32)
    head_w = np.asarray(head_w, np.float32)
    tail_w = np.asarray(tail_w, np.float32)
    tail_b = np.asarray(tail_b, np.float32)

    packed = pack_inputs(state, constants, gammas, head_w, tail_w, tail_b)
    nc = _get_nc()
    in_maps = [{"packed": packed} for _ in range(8)]
    res = run_bass_kernel_spmd(nc, in_maps, core_ids=list(range(8)))
    return res.results[0]["y"].reshape(M).astype(np.float32)


# revision 16
# speedup vs baseline: 1.0069x; 1.0069x over previous
"""Trainium2 Bass kernel for nn_AlgelogicNetwork (fuzzy rule matching -> softmax).

kernel(**inputs) takes the FULL unsharded inputs of reference.setup_inputs()
and returns the FULL output (softmax over M=16 rule strengths, (16,) float32).

The problem is tiny (<<1MB), so the whole computation is replicated on each of
the 8 NeuronCores (SPMD with identical inputs); core 0's output is returned.

v2 design (m = 16 partitions everywhere, minimal serial op count):
  - Host packs all inputs into ONE [16, 120] f32 array (layout ops only:
    reshape/transpose/tile). c and wm are pre-tiled to the flat 36-slot
    (w, j, l) layout so every elementwise op uses <= 2 free dims (HW limit
    for TensorTensor/ScalarTensorTensor); head_w is packed (i,j,l)-ordered so
    the (g>.5) mask broadcast merges into a contiguous jl dim.
  - match~[m,j,w] = sum_l sig[m,j,l]*(c-wm)^2 on Pool: d = crep - wmrep
    (flat), d2 = d*d, ws = d2 * sig (bcast w), match = ws[l=0]+ws[l=1].
  - hww[m,i,j,w] = sum_l (g>.5)*head*wm on DVE; (g>.5)*head is ONE fused
    scalar_tensor_tensor (is_gt then mult).
  - argmin one-hot + gather fused per premise j: scalar_tensor_tensor with the
    per-partition min_j as the scalar operand; j=0 on DVE, j=1 on Pool run in
    parallel; cap[m,i] then falls out of ONE reduce over (j,w) (AxisList XY).
  - tail Linear with bias via [tail|bias] x [cap|1] and a stt+accum_out for
    P2 = |concl|^2; softmax stays in per-partition column form: ACT Sqrt,
    ACT Exp, then S is broadcast to all 16 partitions by a single PE matmul
    whose stationary is the e-column with free-stride 0 (out[a,b] =
    sum_p e[p]*ones[p] = S for every a), recip + mul on DVE, and the output
    DMA gathers the [16,1] column into y[1,16].
  - Bass-init const-AP memsets and the entry all-engine barrier are patched
    out (see _make_bass) so the input DMA issues at ~300ns instead of ~1030ns.
  - Every op keeps the intra-engine own-semaphore discipline for dependent
    same-engine pairs (HW-validated requirement).
"""
import numpy as np
import concourse.bass as bass
from concourse import mybir

F32 = mybir.dt.float32
M, J, I, L, W = 16, 2, 3, 2, 9
FREE = 640
NPACK = 120

# packed columns
C_G, C_HEADI, C_WREP, C_TAILX = 0, 4, 16, 34
C_CAPX, C_ONE, C_BM5, C_ZERO = 42, 45, 46, 47
C_CREP, C_WMR = 48, 84
# computed columns
C_SIG = 120
C_HM = 124
C_T1, C_T2, C_HWW = 136, 190, 244
C_D, C_D2, C_WS = 300, 336, 372
C_MATCH, C_MIN = 408, 426
C_PSEL = 428
C_QA0, C_QA1, C_Q, C_Z, C_ZJ = 482, 498, 514, 530, 546
C_P2, C_P, C_E, C_SINV, C_OUT = 562, 563, 564, 565, 566


def pack_inputs(state, constants, gammas, head_w, tail_w, tail_b):
    p = np.zeros((M, NPACK), np.float32)
    wm = np.asarray(state, np.float32).reshape(W, L)
    p[:, C_G:C_G + 4] = gammas[:, 1:1 + J, :].reshape(M, J * L)
    # head reordered (i, j, l)
    p[:, C_HEADI:C_HEADI + 12] = head_w.transpose(0, 2, 1, 3).reshape(M, I * J * L)
    te = np.concatenate([tail_w, tail_b[:, :, None]], axis=2)        # [16,2,4]
    p[:, C_TAILX:C_TAILX + 8] = te.reshape(M, L * 4)
    p[:, C_ONE] = 1.0
    p[:, C_BM5] = -5.0
    p[:, C_ZERO] = 0.0
    # crep[m, w*4 + j*2 + l] = constants[m, j, l]  (tiled over w)
    c4 = constants[:, :J, :].reshape(M, J * L)
    p[:, C_CREP:C_CREP + 36] = np.tile(c4, (1, W))
    # wmrep36[m, w*4 + j*2 + l] = wm[w, l]  (tiled over j)
    wmr = np.repeat(wm, J, axis=0).reshape(1, W * J * L)             # w,j,l
    p[:, C_WMR:C_WMR + 36] = np.tile(wmr, (M, 1))
    return p


def _make_bass():
    """Bass() with the init-time const-AP memsets and entry all-engine barrier
    suppressed. The four const-AP memsets ([128,1] gpsimd memsets of 0.0/1.0/
    bf16-1.0/u8-127) are only consulted when an activation bias/scale is passed
    as a float immediate - this kernel always passes AP biases, so the const
    APs are never read. The entry barrier only orders engine preambles against
    user code; every op here gates on the input-DMA semaphore (incremented by
    SP after its own preamble), so the barrier is redundant. The Block-exit
    barrier is unaffected (patches are restored before Block exit)."""
    orig_memset = bass.BassSharedVectorInterface.memset
    orig_barrier = bass.Bass.all_engine_barrier
    bass.BassSharedVectorInterface.memset = lambda self, ap, c: None
    bass.Bass.all_engine_barrier = lambda self, sem_only=False: None
    try:
        nc = bass.Bass("TRN2", target_bir_lowering=False, debug=False)
    finally:
        bass.BassSharedVectorInterface.memset = orig_memset
        bass.Bass.all_engine_barrier = orig_barrier
    return nc


def build():
    nc = _make_bass()
    packed = nc.dram_tensor("packed", [M, NPACK], F32, kind="ExternalInput")
    y = nc.dram_tensor("y", [1, 16], F32, kind="ExternalOutput")

    al = mybir.AluOpType
    af = mybir.ActivationFunctionType

    with (
        nc.sbuf_tensor("sb", [128, FREE], F32) as sb,
        nc.psum_tensor("S", [16, 1], F32) as ps,
        nc.semaphore("s_dma") as s_dma,
        nc.semaphore("s_act") as s_act,
        nc.semaphore("s_dve") as s_dve,
        nc.semaphore("s_pe") as s_pe,
        nc.semaphore("s_out") as s_out,
        nc.semaphore("s_pool") as s_pool,
    ):
        def A(r0, nr, c0, dims):
            return bass.AP(sb, r0 * FREE + c0, [[FREE, nr]] + [list(d) for d in dims])

        SPS = lambda: bass.AP(ps, 0, [[1, 16], [1, 1]])

        sems = {"ACT": s_act, "DVE": s_dve, "PE": s_pe, "DMA": s_dma,
                "OUT": s_out, "POOL": s_pool}
        counts = {"ACT": 0, "DVE": 0, "PE": 0, "POOL": 0}
        waited = {k: {} for k in ("ACT", "DVE", "PE", "SP", "POOL")}

        def emit(ekey, engine, build_fn, deps=(), inc=True, own=True):
            # Intra-engine semaphore waits are REQUIRED on this hardware for
            # every DEPENDENT same-engine pair (HW-tested: dropping them
            # corrupts outputs). own=False is legal only when the previous
            # same-engine op is data-independent (disjoint regions; in-order
            # execution suffices) or its completion is transitively implied
            # by one of this op's cross-engine waits (vector-clock join).
            need = {}
            if own and ekey in counts and counts[ekey] > 0:
                need[ekey] = counts[ekey]
            for sk, v in deps:
                if sk == ekey:
                    continue
                need[sk] = max(need.get(sk, 0), v)
            fresh = [(sk, v) for sk, v in need.items() if waited[ekey].get(sk, 0) < v]
            for sk, v in fresh[1:]:
                engine.wait_ge(sems[sk], v)
            inst = build_fn()
            for sk, v in fresh[:1]:
                inst._wait_ge(sems[sk], v)
            for sk, v in fresh:
                waited[ekey][sk] = v
            if inc and ekey in counts:
                counts[ekey] += 1
                inst.then_inc(sems[ekey], 1)
            return inst

        with nc.Block() as block:

            @block.sync
            def _(sync):
                sync.dma_start(
                    out=A(0, M, 0, [(1, NPACK)]),
                    in_=bass.AP(packed, 0, [[NPACK, M], [1, NPACK]]),
                ).then_inc(s_dma, 16)

            @block.scalar
            def _(scalar):
                # a1: sig = sigmoid(10*g - 5)   [16,(j,l)]
                emit("ACT", scalar, lambda: scalar.activation(
                    A(0, M, C_SIG, [(1, 4)]), A(0, M, C_G, [(1, 4)]),
                    af.Sigmoid, bias=A(0, M, C_BM5, [(1, 1)]), scale=10.0,
                ), deps=[("DMA", 16)])

            @block.gpsimd
            def _(g):
                # q0: qa0 = tailx (x) tailx  for l=0   [16,(a,b)]
                emit("POOL", g, lambda: g.tensor_mul(
                    A(0, M, C_QA0, [(4, 4), (1, 4)]),
                    A(0, M, C_TAILX, [(1, 4), (0, 4)]),
                    A(0, M, C_TAILX, [(0, 4), (1, 4)]),
                ), deps=[("DMA", 16)])
                # t1 = hm[l=0] * wm[l=0]   [16,(ij,w)]
                emit("POOL", g, lambda: g.tensor_mul(
                    A(0, M, C_T1, [(9, 6), (1, 9)]),
                    A(0, M, C_HM, [(2, 6), (0, 9)]),
                    A(0, M, C_WMR, [(0, 6), (4, 9)]),
                ), deps=[("DVE", 1)], own=False)
                # t2 = hm[l=1] * wm[l=1]  (independent of t1; in-order)
                emit("POOL", g, lambda: g.tensor_mul(
                    A(0, M, C_T2, [(9, 6), (1, 9)]),
                    A(0, M, C_HM + 1, [(2, 6), (0, 9)]),
                    A(0, M, C_WMR + 1, [(0, 6), (4, 9)]),
                ), own=False)
                # hww = t1 + t2   [16,(i,j,w)]
                emit("POOL", g, lambda: g.tensor_add(
                    A(0, M, C_HWW, [(1, 54)]), A(0, M, C_T1, [(1, 54)]),
                    A(0, M, C_T2, [(1, 54)]),
                ))
                # qa1 (independent of hww; in-order after it)
                emit("POOL", g, lambda: g.tensor_mul(
                    A(0, M, C_QA1, [(4, 4), (1, 4)]),
                    A(0, M, C_TAILX + 4, [(1, 4), (0, 4)]),
                    A(0, M, C_TAILX + 4, [(0, 4), (1, 4)]),
                ), own=False)
                # Q = qa0 + qa1 = sum_l tailx_l (x) tailx_l   [16,(a,b)]
                emit("POOL", g, lambda: g.tensor_add(
                    A(0, M, C_Q, [(1, 16)]), A(0, M, C_QA0, [(1, 16)]),
                    A(0, M, C_QA1, [(1, 16)]),
                ))

            @block.vector
            def _(vector):
                # v1: hm = (g > 0.5) * head   [16,(i,jl)]
                emit("DVE", vector, lambda: vector.scalar_tensor_tensor(
                    A(0, M, C_HM, [(4, 3), (1, 4)]),
                    A(0, M, C_G, [(0, 3), (1, 4)]),
                    0.5,
                    A(0, M, C_HEADI, [(4, 3), (1, 4)]),
                    op0=al.is_gt, op1=al.mult,
                ), deps=[("DMA", 16)])
                # v2: d = crep - wmrep   [16, 36]  (indep of hm)
                emit("DVE", vector, lambda: vector.tensor_tensor(
                    A(0, M, C_D, [(1, 36)]), A(0, M, C_CREP, [(1, 36)]),
                    A(0, M, C_WMR, [(1, 36)]), al.subtract,
                ), own=False)
                # v3: d2 = d*d  (dep d = inc 2)
                emit("DVE", vector, lambda: vector.tensor_mul(
                    A(0, M, C_D2, [(1, 36)]), A(0, M, C_D, [(1, 36)]),
                    A(0, M, C_D, [(1, 36)]),
                ), deps=[("DVE", 2)], own=False)
                # v4: ws = d2 * sig (bcast w; jl contiguous)
                emit("DVE", vector, lambda: vector.tensor_mul(
                    A(0, M, C_WS, [(4, 9), (1, 4)]),
                    A(0, M, C_D2, [(4, 9), (1, 4)]),
                    A(0, M, C_SIG, [(0, 9), (1, 4)]),
                ), deps=[("ACT", 1)])
                # v5: match = ws[l=0] + ws[l=1]   [16,(j,w)]
                emit("DVE", vector, lambda: vector.tensor_add(
                    A(0, M, C_MATCH, [(9, 2), (1, 9)]),
                    A(0, M, C_WS, [(2, 2), (4, 9)]),
                    A(0, M, C_WS + 1, [(2, 2), (4, 9)]),
                ))
                # v6: min over w per j    [16,(j)]
                emit("DVE", vector, lambda: vector.tensor_reduce(
                    A(0, M, C_MIN, [(1, 2)]),
                    A(0, M, C_MATCH, [(9, 2), (1, 9)]),
                    axis=mybir.AxisListType.X, op=al.min,
                ))
                # v7: psel_j0 = (match_0 == min_0) * hww_0   [16,(i,w)]
                emit("DVE", vector, lambda: vector.scalar_tensor_tensor(
                    A(0, M, C_PSEL, [(18, 3), (1, 9)]),
                    A(0, M, C_MATCH, [(0, 3), (1, 9)]),
                    A(0, M, C_MIN, [(1, 1)]),
                    A(0, M, C_HWW, [(18, 3), (1, 9)]),
                    op0=al.is_equal, op1=al.mult,
                ), deps=[("POOL", 4)])
                # v8: psel_j1 (deps covered by v7's waits; in-order)
                emit("DVE", vector, lambda: vector.scalar_tensor_tensor(
                    A(0, M, C_PSEL + 9, [(18, 3), (1, 9)]),
                    A(0, M, C_MATCH + 9, [(0, 3), (1, 9)]),
                    A(0, M, C_MIN + 1, [(1, 1)]),
                    A(0, M, C_HWW + 9, [(18, 3), (1, 9)]),
                    op0=al.is_equal, op1=al.mult,
                ), own=False)
                # v9: cap = sum_{j,w} psel   [16,(i)]
                emit("DVE", vector, lambda: vector.tensor_reduce(
                    A(0, M, C_CAPX, [(1, 3)]),
                    A(0, M, C_PSEL, [(18, 3), (9, 2), (1, 9)]),
                    axis=mybir.AxisListType.XY, op=al.add,
                ))
                # v10: z = capx (x) capx   [16,(a,b)]
                emit("DVE", vector, lambda: vector.tensor_mul(
                    A(0, M, C_Z, [(4, 4), (1, 4)]),
                    A(0, M, C_CAPX, [(1, 4), (0, 4)]),
                    A(0, M, C_CAPX, [(0, 4), (1, 4)]),
                ))
                # v11: P2 = sum_{a,b} z*Q  (= |T capx|^2, bias included)
                emit("DVE", vector, lambda: vector.scalar_tensor_tensor(
                    A(0, M, C_ZJ, [(1, 16)]), A(0, M, C_Z, [(1, 16)]), 1.0,
                    A(0, M, C_Q, [(1, 16)]), op0=al.mult, op1=al.mult,
                    accum_out=A(0, M, C_P2, [(1, 1)]),
                ), deps=[("POOL", 6)])

            @block.scalar
            def _(scalar):
                # a2: P = sqrt(P2)   [16,1]
                emit("ACT", scalar, lambda: scalar.activation(
                    A(0, M, C_P, [(1, 1)]), A(0, M, C_P2, [(1, 1)]),
                    af.Sqrt, bias=A(0, M, C_ZERO, [(1, 1)]), scale=1.0,
                ), deps=[("DVE", 11)], own=False)
                # a3: e = exp(P)   [16,1]
                emit("ACT", scalar, lambda: scalar.activation(
                    A(0, M, C_E, [(1, 1)]), A(0, M, C_P, [(1, 1)]),
                    af.Exp, bias=A(0, M, C_ZERO, [(1, 1)]), scale=1.0,
                ))

            @block.tensor
            def _(tensor):
                # m1: S = sum_p e[p], broadcast to 16 partitions via
                # free-stride-0 stationary: out[a,0] = sum_p e[p]*one[p]
                emit("PE", tensor, lambda: tensor.matmul(
                    SPS(),
                    bass.AP(sb, C_E, [[FREE, 16], [0, 16]]),
                    A(0, 16, C_ONE, [(1, 1)]),
                    start=True, stop=True,
                ), deps=[("ACT", 3)])

            @block.vector
            def _(vector):
                # v12: sinv = 1/S (PSUM -> SBUF)
                emit("DVE", vector, lambda: vector.reciprocal(
                    A(0, M, C_SINV, [(1, 1)]), SPS(),
                ), deps=[("PE", 1)], own=False)
                # v13: out = e * sinv
                emit("DVE", vector, lambda: vector.tensor_scalar(
                    A(0, M, C_OUT, [(1, 1)]), A(0, M, C_E, [(1, 1)]),
                    A(0, M, C_SINV, [(1, 1)]), None, al.mult,
                ))

            @block.sync
            def _(sync):
                emit("SP", sync, lambda: sync.dma_start(
                    out=bass.AP(y, 0, [[16, 1], [1, 16]]),
                    in_=A(0, M, C_OUT, [(1, 1)]),
                ), deps=[("DVE", 13)], inc=False).then_inc(s_out, 16)

    return nc


_NC = None


def _get_nc():
    global _NC
    if _NC is None:
        _NC = build()
    return _NC


def _default_inputs():
    """Regenerate setup_inputs()'s non-state parameters (jax key(0) recipe) in
    case the harness only supplies `state` (spec.json lists only state in
    input_specs)."""
    import jax
    import jax.numpy as jnp
    key = jax.random.key(0)
    ks = jax.random.split(key, 6)
    bL = 1.0 / np.sqrt(L)
    bI = 1.0 / np.sqrt(I)
    return dict(
        state=jax.random.normal(ks[0], (1, W * L), dtype=jnp.float32),
        constants=jax.random.uniform(ks[1], (M, J + 1, L), minval=-1.0, maxval=1.0, dtype=jnp.float32),
        gammas=jax.random.uniform(ks[2], (M, J + 1, L), minval=0.0, maxval=1.0, dtype=jnp.float32),
        head_w=jax.random.uniform(ks[3], (M, J, I, L), minval=-bL, maxval=bL, dtype=jnp.float32),
        tail_w=jax.random.uniform(ks[4], (M, L, I), minval=-bI, maxval=bI, dtype=jnp.float32),
        tail_b=jax.random.uniform(ks[5], (M, L), minval=-bI, maxval=bI, dtype=jnp.float32),
    )


def kernel(state=None, constants=None, gammas=None, head_w=None, tail_w=None,
           tail_b=None, **_unused):
    from concourse.bass_utils import run_bass_kernel_spmd

    if any(v is None for v in (state, constants, gammas, head_w, tail_w, tail_b)):
        d = _default_inputs()
        state = d["state"] if state is None else state
        constants = d["constants"] if constants is None else constants
        gammas = d["gammas"] if gammas is None else gammas
        head_w = d["head_w"] if head_w is None else head_w
        tail_w = d["tail_w"] if tail_w is None else tail_w
        tail_b = d["tail_b"] if tail_b is None else tail_b

    state = np.asarray(state, np.float32)
    constants = np.asarray(constants, np.float32)
    gammas = np.asarray(gammas, np.float32)
    head_w = np.asarray(head_w, np.float32)
    tail_w = np.asarray(tail_w, np.float32)
    tail_b = np.asarray(tail_b, np.float32)

    packed = pack_inputs(state, constants, gammas, head_w, tail_w, tail_b)
    nc = _get_nc()
    in_maps = [{"packed": packed} for _ in range(8)]
    res = run_bass_kernel_spmd(nc, in_maps, core_ids=list(range(8)))
    return res.results[0]["y"].reshape(M).astype(np.float32)


# revision 17
# speedup vs baseline: 1.0142x; 1.0072x over previous
"""Trainium2 Bass kernel for nn_AlgelogicNetwork (fuzzy rule matching -> softmax).

kernel(**inputs) takes the FULL unsharded inputs of reference.setup_inputs()
and returns the FULL output (softmax over M=16 rule strengths, (16,) float32).

The problem is tiny (<<1MB), so the whole computation is replicated on each of
the 8 NeuronCores (SPMD with identical inputs); core 0's output is returned.

v2 design (m = 16 partitions everywhere, minimal serial op count):
  - Host packs all inputs into ONE [16, 120] f32 array (layout ops only:
    reshape/transpose/tile). c and wm are pre-tiled to the flat 36-slot
    (w, j, l) layout so every elementwise op uses <= 2 free dims (HW limit
    for TensorTensor/ScalarTensorTensor); head_w is packed (i,j,l)-ordered so
    the (g>.5) mask broadcast merges into a contiguous jl dim.
  - match~[m,j,w] = sum_l sig[m,j,l]*(c-wm)^2 on Pool: d = crep - wmrep
    (flat), d2 = d*d, ws = d2 * sig (bcast w), match = ws[l=0]+ws[l=1].
  - hww[m,i,j,w] = sum_l (g>.5)*head*wm on DVE; (g>.5)*head is ONE fused
    scalar_tensor_tensor (is_gt then mult).
  - argmin one-hot + gather fused per premise j: scalar_tensor_tensor with the
    per-partition min_j as the scalar operand; j=0 on DVE, j=1 on Pool run in
    parallel; cap[m,i] then falls out of ONE reduce over (j,w) (AxisList XY).
  - tail Linear with bias via [tail|bias] x [cap|1] and a stt+accum_out for
    P2 = |concl|^2; softmax stays in per-partition column form: ACT Sqrt,
    ACT Exp, then S is broadcast to all 16 partitions by a single PE matmul
    whose stationary is the e-column with free-stride 0 (out[a,b] =
    sum_p e[p]*ones[p] = S for every a), recip + mul on DVE, and the output
    DMA gathers the [16,1] column into y[1,16].
  - Bass-init const-AP memsets and the entry all-engine barrier are patched
    out (see _make_bass) so the input DMA issues at ~300ns instead of ~1030ns.
  - Every op keeps the intra-engine own-semaphore discipline for dependent
    same-engine pairs (HW-validated requirement).
"""
import numpy as np
import concourse.bass as bass
from concourse import mybir

F32 = mybir.dt.float32
M, J, I, L, W = 16, 2, 3, 2, 9
FREE = 640
NPACK = 120

# packed columns
C_G, C_HEADI, C_WREP, C_TAILX = 0, 4, 16, 34
C_CAPX, C_ONE, C_BM5, C_ZERO = 42, 45, 46, 47
C_CREP, C_WMR = 48, 84
# computed columns
C_SIG = 120
C_HM = 124
C_T1, C_T2, C_HWW = 136, 190, 244
C_D, C_D2, C_WS = 300, 336, 372
C_MATCH, C_MIN = 408, 426
C_PSEL = 428
C_QA0, C_QA1, C_Q, C_Z, C_ZJ = 482, 498, 514, 530, 546
C_P2, C_P, C_E, C_SINV, C_OUT = 562, 563, 564, 565, 566


def pack_inputs(state, constants, gammas, head_w, tail_w, tail_b):
    p = np.zeros((M, NPACK), np.float32)
    wm = np.asarray(state, np.float32).reshape(W, L)
    p[:, C_G:C_G + 4] = gammas[:, 1:1 + J, :].reshape(M, J * L)
    # head reordered (i, j, l)
    p[:, C_HEADI:C_HEADI + 12] = head_w.transpose(0, 2, 1, 3).reshape(M, I * J * L)
    te = np.concatenate([tail_w, tail_b[:, :, None]], axis=2)        # [16,2,4]
    p[:, C_TAILX:C_TAILX + 8] = te.reshape(M, L * 4)
    p[:, C_ONE] = 1.0
    p[:, C_BM5] = -5.0
    p[:, C_ZERO] = 0.0
    # crep[m, w*4 + j*2 + l] = constants[m, j, l]  (tiled over w)
    c4 = constants[:, :J, :].reshape(M, J * L)
    p[:, C_CREP:C_CREP + 36] = np.tile(c4, (1, W))
    # wmrep36[m, w*4 + j*2 + l] = wm[w, l]  (tiled over j)
    wmr = np.repeat(wm, J, axis=0).reshape(1, W * J * L)             # w,j,l
    p[:, C_WMR:C_WMR + 36] = np.tile(wmr, (M, 1))
    return p


def _make_bass():
    """Bass() with the init-time const-AP memsets and entry all-engine barrier
    suppressed. The four const-AP memsets ([128,1] gpsimd memsets of 0.0/1.0/
    bf16-1.0/u8-127) are only consulted when an activation bias/scale is passed
    as a float immediate - this kernel always passes AP biases, so the const
    APs are never read. The entry barrier only orders engine preambles against
    user code; every op here gates on the input-DMA semaphore (incremented by
    SP after its own preamble), so the barrier is redundant. The Block-exit
    barrier is unaffected (patches are restored before Block exit)."""
    orig_memset = bass.BassSharedVectorInterface.memset
    orig_barrier = bass.Bass.all_engine_barrier
    bass.BassSharedVectorInterface.memset = lambda self, ap, c: None
    bass.Bass.all_engine_barrier = lambda self, sem_only=False: None
    try:
        nc = bass.Bass("TRN2", target_bir_lowering=False, debug=False)
    finally:
        bass.BassSharedVectorInterface.memset = orig_memset
        bass.Bass.all_engine_barrier = orig_barrier
    return nc


def build():
    nc = _make_bass()
    packed = nc.dram_tensor("packed", [M, NPACK], F32, kind="ExternalInput")
    y = nc.dram_tensor("y", [1, 16], F32, kind="ExternalOutput")

    al = mybir.AluOpType
    af = mybir.ActivationFunctionType

    with (
        nc.sbuf_tensor("sb", [128, FREE], F32) as sb,
        nc.psum_tensor("S", [16, 1], F32) as ps,
        nc.semaphore("s_dma") as s_dma,
        nc.semaphore("s_act") as s_act,
        nc.semaphore("s_dve") as s_dve,
        nc.semaphore("s_pe") as s_pe,
        nc.semaphore("s_out") as s_out,
        nc.semaphore("s_pool") as s_pool,
    ):
        def A(r0, nr, c0, dims):
            return bass.AP(sb, r0 * FREE + c0, [[FREE, nr]] + [list(d) for d in dims])

        SPS = lambda: bass.AP(ps, 0, [[1, 16], [1, 1]])

        sems = {"ACT": s_act, "DVE": s_dve, "PE": s_pe, "DMA": s_dma,
                "OUT": s_out, "POOL": s_pool}
        counts = {"ACT": 0, "DVE": 0, "PE": 0, "POOL": 0}
        waited = {k: {} for k in ("ACT", "DVE", "PE", "SP", "POOL")}

        def emit(ekey, engine, build_fn, deps=(), inc=True, own=True):
            # Intra-engine semaphore waits are REQUIRED on this hardware for
            # every DEPENDENT same-engine pair (HW-tested: dropping them
            # corrupts outputs). own=False is legal only when the previous
            # same-engine op is data-independent (disjoint regions; in-order
            # execution suffices) or its completion is transitively implied
            # by one of this op's cross-engine waits (vector-clock join).
            need = {}
            if own and ekey in counts and counts[ekey] > 0:
                need[ekey] = counts[ekey]
            for sk, v in deps:
                if sk == ekey:
                    continue
                need[sk] = max(need.get(sk, 0), v)
            fresh = [(sk, v) for sk, v in need.items() if waited[ekey].get(sk, 0) < v]
            for sk, v in fresh[1:]:
                engine.wait_ge(sems[sk], v)
            inst = build_fn()
            for sk, v in fresh[:1]:
                inst._wait_ge(sems[sk], v)
            for sk, v in fresh:
                waited[ekey][sk] = v
            if inc and ekey in counts:
                counts[ekey] += 1
                inst.then_inc(sems[ekey], 1)
            return inst

        nc.sync.dma_start(
            out=A(0, M, 0, [(1, NPACK)]),
            in_=bass.AP(packed, 0, [[NPACK, M], [1, NPACK]]),
        ).then_inc(s_dma, 16)

        with nc.Block() as block:

            @block.scalar
            def _(scalar):
                # a1: sig = sigmoid(10*g - 5)   [16,(j,l)]
                emit("ACT", scalar, lambda: scalar.activation(
                    A(0, M, C_SIG, [(1, 4)]), A(0, M, C_G, [(1, 4)]),
                    af.Sigmoid, bias=A(0, M, C_BM5, [(1, 1)]), scale=10.0,
                ), deps=[("DMA", 16)])

            @block.gpsimd
            def _(g):
                # q0: qa0 = tailx (x) tailx  for l=0   [16,(a,b)]
                emit("POOL", g, lambda: g.tensor_mul(
                    A(0, M, C_QA0, [(4, 4), (1, 4)]),
                    A(0, M, C_TAILX, [(1, 4), (0, 4)]),
                    A(0, M, C_TAILX, [(0, 4), (1, 4)]),
                ), deps=[("DMA", 16)])
                # t1 = hm[l=0] * wm[l=0]   [16,(ij,w)]
                emit("POOL", g, lambda: g.tensor_mul(
                    A(0, M, C_T1, [(9, 6), (1, 9)]),
                    A(0, M, C_HM, [(2, 6), (0, 9)]),
                    A(0, M, C_WMR, [(0, 6), (4, 9)]),
                ), deps=[("DVE", 1)], own=False)
                # t2 = hm[l=1] * wm[l=1]  (independent of t1; in-order)
                emit("POOL", g, lambda: g.tensor_mul(
                    A(0, M, C_T2, [(9, 6), (1, 9)]),
                    A(0, M, C_HM + 1, [(2, 6), (0, 9)]),
                    A(0, M, C_WMR + 1, [(0, 6), (4, 9)]),
                ), own=False)
                # hww = t1 + t2   [16,(i,j,w)]
                emit("POOL", g, lambda: g.tensor_add(
                    A(0, M, C_HWW, [(1, 54)]), A(0, M, C_T1, [(1, 54)]),
                    A(0, M, C_T2, [(1, 54)]),
                ))
                # qa1 (independent of hww; in-order after it)
                emit("POOL", g, lambda: g.tensor_mul(
                    A(0, M, C_QA1, [(4, 4), (1, 4)]),
                    A(0, M, C_TAILX + 4, [(1, 4), (0, 4)]),
                    A(0, M, C_TAILX + 4, [(0, 4), (1, 4)]),
                ), own=False)
                # Q = qa0 + qa1 = sum_l tailx_l (x) tailx_l   [16,(a,b)]
                emit("POOL", g, lambda: g.tensor_add(
                    A(0, M, C_Q, [(1, 16)]), A(0, M, C_QA0, [(1, 16)]),
                    A(0, M, C_QA1, [(1, 16)]),
                ))

            @block.vector
            def _(vector):
                # v1: hm = (g > 0.5) * head   [16,(i,jl)]
                emit("DVE", vector, lambda: vector.scalar_tensor_tensor(
                    A(0, M, C_HM, [(4, 3), (1, 4)]),
                    A(0, M, C_G, [(0, 3), (1, 4)]),
                    0.5,
                    A(0, M, C_HEADI, [(4, 3), (1, 4)]),
                    op0=al.is_gt, op1=al.mult,
                ), deps=[("DMA", 16)])
                # v2: d = crep - wmrep   [16, 36]  (indep of hm)
                emit("DVE", vector, lambda: vector.tensor_tensor(
                    A(0, M, C_D, [(1, 36)]), A(0, M, C_CREP, [(1, 36)]),
                    A(0, M, C_WMR, [(1, 36)]), al.subtract,
                ), own=False)
                # v3: d2 = d*d  (dep d = inc 2)
                emit("DVE", vector, lambda: vector.tensor_mul(
                    A(0, M, C_D2, [(1, 36)]), A(0, M, C_D, [(1, 36)]),
                    A(0, M, C_D, [(1, 36)]),
                ), deps=[("DVE", 2)], own=False)
                # v4: ws = d2 * sig (bcast w; jl contiguous)
                emit("DVE", vector, lambda: vector.tensor_mul(
                    A(0, M, C_WS, [(4, 9), (1, 4)]),
                    A(0, M, C_D2, [(4, 9), (1, 4)]),
                    A(0, M, C_SIG, [(0, 9), (1, 4)]),
                ), deps=[("ACT", 1)])
                # v5: match = ws[l=0] + ws[l=1]   [16,(j,w)]
                emit("DVE", vector, lambda: vector.tensor_add(
                    A(0, M, C_MATCH, [(9, 2), (1, 9)]),
                    A(0, M, C_WS, [(2, 2), (4, 9)]),
                    A(0, M, C_WS + 1, [(2, 2), (4, 9)]),
                ))
                # v6: min over w per j    [16,(j)]
                emit("DVE", vector, lambda: vector.tensor_reduce(
                    A(0, M, C_MIN, [(1, 2)]),
                    A(0, M, C_MATCH, [(9, 2), (1, 9)]),
                    axis=mybir.AxisListType.X, op=al.min,
                ))
                # v7: psel_j0 = (match_0 == min_0) * hww_0   [16,(i,w)]
                emit("DVE", vector, lambda: vector.scalar_tensor_tensor(
                    A(0, M, C_PSEL, [(18, 3), (1, 9)]),
                    A(0, M, C_MATCH, [(0, 3), (1, 9)]),
                    A(0, M, C_MIN, [(1, 1)]),
                    A(0, M, C_HWW, [(18, 3), (1, 9)]),
                    op0=al.is_equal, op1=al.mult,
                ), deps=[("POOL", 4)])
                # v8: psel_j1 (deps covered by v7's waits; in-order)
                emit("DVE", vector, lambda: vector.scalar_tensor_tensor(
                    A(0, M, C_PSEL + 9, [(18, 3), (1, 9)]),
                    A(0, M, C_MATCH + 9, [(0, 3), (1, 9)]),
                    A(0, M, C_MIN + 1, [(1, 1)]),
                    A(0, M, C_HWW + 9, [(18, 3), (1, 9)]),
                    op0=al.is_equal, op1=al.mult,
                ), own=False)
                # v9: cap = sum_{j,w} psel   [16,(i)]
                emit("DVE", vector, lambda: vector.tensor_reduce(
                    A(0, M, C_CAPX, [(1, 3)]),
                    A(0, M, C_PSEL, [(18, 3), (9, 2), (1, 9)]),
                    axis=mybir.AxisListType.XY, op=al.add,
                ))
                # v10: z = capx (x) capx   [16,(a,b)]
                emit("DVE", vector, lambda: vector.tensor_mul(
                    A(0, M, C_Z, [(4, 4), (1, 4)]),
                    A(0, M, C_CAPX, [(1, 4), (0, 4)]),
                    A(0, M, C_CAPX, [(0, 4), (1, 4)]),
                ))
                # v11: P2 = sum_{a,b} z*Q  (= |T capx|^2, bias included)
                emit("DVE", vector, lambda: vector.scalar_tensor_tensor(
                    A(0, M, C_ZJ, [(1, 16)]), A(0, M, C_Z, [(1, 16)]), 1.0,
                    A(0, M, C_Q, [(1, 16)]), op0=al.mult, op1=al.mult,
                    accum_out=A(0, M, C_P2, [(1, 1)]),
                ), deps=[("POOL", 6)])

            @block.scalar
            def _(scalar):
                # a2: P = sqrt(P2)   [16,1]
                emit("ACT", scalar, lambda: scalar.activation(
                    A(0, M, C_P, [(1, 1)]), A(0, M, C_P2, [(1, 1)]),
                    af.Sqrt, bias=A(0, M, C_ZERO, [(1, 1)]), scale=1.0,
                ), deps=[("DVE", 11)], own=False)
                # a3: e = exp(P)   [16,1]
                emit("ACT", scalar, lambda: scalar.activation(
                    A(0, M, C_E, [(1, 1)]), A(0, M, C_P, [(1, 1)]),
                    af.Exp, bias=A(0, M, C_ZERO, [(1, 1)]), scale=1.0,
                ))

            @block.tensor
            def _(tensor):
                # m1: S = sum_p e[p], broadcast to 16 partitions via
                # free-stride-0 stationary: out[a,0] = sum_p e[p]*one[p]
                emit("PE", tensor, lambda: tensor.matmul(
                    SPS(),
                    bass.AP(sb, C_E, [[FREE, 16], [0, 16]]),
                    A(0, 16, C_ONE, [(1, 1)]),
                    start=True, stop=True,
                ), deps=[("ACT", 3)])

            @block.vector
            def _(vector):
                # v12: sinv = 1/S (PSUM -> SBUF)
                emit("DVE", vector, lambda: vector.reciprocal(
                    A(0, M, C_SINV, [(1, 1)]), SPS(),
                ), deps=[("PE", 1)], own=False)
                # v13: out = e * sinv
                emit("DVE", vector, lambda: vector.tensor_scalar(
                    A(0, M, C_OUT, [(1, 1)]), A(0, M, C_E, [(1, 1)]),
                    A(0, M, C_SINV, [(1, 1)]), None, al.mult,
                ))

            @block.sync
            def _(sync):
                emit("SP", sync, lambda: sync.dma_start(
                    out=bass.AP(y, 0, [[16, 1], [1, 16]]),
                    in_=A(0, M, C_OUT, [(1, 1)]),
                ), deps=[("DVE", 13)], inc=False).then_inc(s_out, 16)

    return nc


_NC = None


def _get_nc():
    global _NC
    if _NC is None:
        _NC = build()
    return _NC


def _default_inputs():
    """Regenerate setup_inputs()'s non-state parameters (jax key(0) recipe) in
    case the harness only supplies `state` (spec.json lists only state in
    input_specs)."""
    import jax
    import jax.numpy as jnp
    key = jax.random.key(0)
    ks = jax.random.split(key, 6)
    bL = 1.0 / np.sqrt(L)
    bI = 1.0 / np.sqrt(I)
    return dict(
        state=jax.random.normal(ks[0], (1, W * L), dtype=jnp.float32),
        constants=jax.random.uniform(ks[1], (M, J + 1, L), minval=-1.0, maxval=1.0, dtype=jnp.float32),
        gammas=jax.random.uniform(ks[2], (M, J + 1, L), minval=0.0, maxval=1.0, dtype=jnp.float32),
        head_w=jax.random.uniform(ks[3], (M, J, I, L), minval=-bL, maxval=bL, dtype=jnp.float32),
        tail_w=jax.random.uniform(ks[4], (M, L, I), minval=-bI, maxval=bI, dtype=jnp.float32),
        tail_b=jax.random.uniform(ks[5], (M, L), minval=-bI, maxval=bI, dtype=jnp.float32),
    )


def kernel(state=None, constants=None, gammas=None, head_w=None, tail_w=None,
           tail_b=None, **_unused):
    from concourse.bass_utils import run_bass_kernel_spmd

    if any(v is None for v in (state, constants, gammas, head_w, tail_w, tail_b)):
        d = _default_inputs()
        state = d["state"] if state is None else state
        constants = d["constants"] if constants is None else constants
        gammas = d["gammas"] if gammas is None else gammas
        head_w = d["head_w"] if head_w is None else head_w
        tail_w = d["tail_w"] if tail_w is None else tail_w
        tail_b = d["tail_b"] if tail_b is None else tail_b

    state = np.asarray(state, np.float32)
    constants = np.asarray(constants, np.float32)
    gammas = np.asarray(gammas, np.float32)
    head_w = np.asarray(head_w, np.float32)
    tail_w = np.asarray(tail_w, np.float32)
    tail_b = np.asarray(tail_b, np.float32)

    packed = pack_inputs(state, constants, gammas, head_w, tail_w, tail_b)
    nc = _get_nc()
    in_maps = [{"packed": packed} for _ in range(8)]
    res = run_bass_kernel_spmd(nc, in_maps, core_ids=list(range(8)))
    return res.results[0]["y"].reshape(M).astype(np.float32)
